# revision 1
# baseline (speedup 1.0000x reference)
"""DGConv2 (dynamic-graph edge conv) Trainium2 kernel, 8-core SPMD.

Contract: kernel(**inputs) takes the FULL inputs of the nn module
(x [4,64,8192] f32, W1 [128,128], W2 [128,128], spiral_size=20) and returns
the FULL output [4,128,8192] f32. Sharding: data-parallel over (batch b,
query half h) -> 8 shards; each NeuronCore computes the output columns for
4096 queries of one batch, with the batch's full point set as candidates.

Per-core algorithm (see validate notes):
  s[q,m] = 2 x_q.x_m - ||x_m||^2                (ranking == reference knn)
  top-20 exact via vector-engine max8/max_index/match_replace rounds
  softmax-over-K gate == softmax(A[:, idx]) with A = W1[:64,:64] @ x
  (all per-query terms cancel in the softmax; sum-of-gates == 1 collapses
   the center half of both convs into precomputable matmuls)
  out = W2[:,:64] @ (seg_sum(E*Xg)/seg_sum(E)) + (W2[:,64:]-W2[:,:64]) @ x_q
"""

import numpy as np

import concourse.bacc as bacc
import concourse.tile as tile
import concourse.mybir as mybir
import concourse.bass_utils as bass_utils
from concourse import bass2jax
from concourse.bass_interp import get_hw_module
from concourse.masks import make_identity

F32 = mybir.dt.float32
I16 = mybir.dt.int16
U16 = mybir.dt.uint16
AF = mybir.ActivationFunctionType
ALU = mybir.AluOpType
AX = mybir.AxisListType

B, C, N, O, K = 4, 64, 8192, 128, 20
TWO_C = 2 * C
N_CORES = 8
Q = N * B // N_CORES            # 4096 queries per core
QT = 128                        # queries per tile
CHUNK = 512                     # matmul free-dim chunk
NEG = -1.0e30


def _build(tc, outs, ins, n, q):
    nc = tc.nc
    n_chunks = n // CHUNK
    n_tiles = q // QT
    nki = QT * K                                   # gather idx per tile

    XB, XQA, XT = ins["XB"], ins["XQA"], ins["XT"]
    W1AAT, W2AT, W2BT = ins["W1AAT"], ins["W2AT"], ins["W2BT"]
    OUT = outs["OUT"]

    TP = nc.dram_tensor("TP_scratch", [n, TWO_C], F32, kind="Internal").ap()
    O2D = nc.dram_tensor("O2_scratch", [O, q], F32, kind="Internal").ap()

    with (
        tc.tile_pool(name="persist", bufs=1) as pp,
        tc.tile_pool(name="pD", bufs=2) as pD,
        tc.tile_pool(name="pwork", bufs=2) as pw,
        tc.tile_pool(name="pidxd", bufs=2, space="DRAM") as pdram,
        tc.tile_pool(name="ps_d", bufs=4, space="PSUM") as psd,
        tc.tile_pool(name="ps_s", bufs=2, space="PSUM") as pss,
    ):
        Ms = pp.tile([C + 1, n], F32, tag="Ms")      # rows 0-63 x, row 64 xx
        XQAs = pp.tile([C + 1, q], F32, tag="XQAs")
        W1s = pp.tile([C, C], F32, tag="W1s")
        W2as = pp.tile([C, O], F32, tag="W2as")
        W2bs = pp.tile([C, O], F32, tag="W2bs")
        W2ds = pp.tile([C, O], F32, tag="W2ds")
        ident = pp.tile([128, 128], F32, tag="ident")
        ones = pp.tile([C, 128], F32, tag="ones")

        nc.sync.dma_start(out=Ms[0:C, :], in_=XB)
        nc.sync.dma_start(out=XQAs, in_=XQA)
        nc.sync.dma_start(out=W1s, in_=W1AAT)
        nc.sync.dma_start(out=W2as, in_=W2AT)
        nc.sync.dma_start(out=W2bs, in_=W2BT)
        make_identity(nc, ident)
        nc.vector.memset(ones, 1.0)
        nc.vector.tensor_sub(W2ds, W2bs, W2as)
        nc.vector.tensor_scalar_mul(W2ds, W2ds, 0.5)   # XQA rows are 2*x_q

        # ---- one-time setup: xx row of Ms, gather table TP, O2 ----
        with tc.tile_pool(name="psetup", bufs=2) as pset:
            XSQ = pp.tile([C, n], F32, tag="XSQ")
            nc.scalar.activation(XSQ, Ms[0:C, :], AF.Square)
            for ch in range(n_chunks):
                ps = psd.tile([128, CHUNK], F32, tag="pd")
                nc.tensor.matmul(ps, ones, XSQ[:, ch * CHUNK:(ch + 1) * CHUNK],
                                 start=True, stop=True)
                nc.scalar.copy(Ms[C:C + 1, ch * CHUNK:(ch + 1) * CHUNK],
                               ps[C:C + 1, :])

            nc.sync.dma_start(out=TP[:, 0:C], in_=XT)
            for j in range(n // 128):
                psa_t = pss.tile([128, 128], F32, tag="po")
                psa = psa_t[:, 0:C]
                nc.tensor.matmul(psa, Ms[0:C, j * 128:(j + 1) * 128], W1s,
                                 start=True, stop=True)
                te = pset.tile([128, C], F32, tag="te")
                nc.scalar.activation(te, psa, AF.Exp)
                nc.sync.dma_start(out=TP[j * 128:(j + 1) * 128, C:TWO_C], in_=te)

            oc = min(CHUNK, q)
            for ch in range(q // oc):
                ps2 = psd.tile([128, CHUNK], F32, tag="pd")
                nc.tensor.matmul(ps2[:, 0:oc], W2ds, XQAs[0:C, ch * oc:(ch + 1) * oc],
                                 start=True, stop=True)
                to2 = pset.tile([128, CHUNK], F32, tag="to2")
                nc.scalar.copy(to2[:, 0:oc], ps2[:, 0:oc])
                nc.sync.dma_start(out=O2D[:, ch * oc:(ch + 1) * oc], in_=to2[:, 0:oc])

        # ---- main loop over query tiles (1-tile software-pipeline skew:
        #      tile t's idx-DMA+gather overlaps tile t+1's DVE extraction) ----
        tg_slots = {}

        def tile_front(t):
            q0 = t * QT
            Ds = pD.tile([128, n], F32, tag="Ds")
            for ch in range(n_chunks):
                pd = psd.tile([128, CHUNK], F32, tag="pd")
                nc.tensor.matmul(pd, XQAs[:, q0:q0 + QT],
                                 Ms[:, ch * CHUNK:(ch + 1) * CHUNK],
                                 start=True, stop=True)
                nc.scalar.copy(Ds[:, ch * CHUNK:(ch + 1) * CHUNK], pd)

            # exact top-20: 3 rounds of (max8, max_index, match_replace)
            Vall = pw.tile([128, 24], F32, tag="Vall")
            Iall = pw.tile([128, 24], U16, tag="Iall")
            for r in range(3):
                nc.vector.max(out=Vall[:, 8 * r:8 * r + 8], in_=Ds)
                nc.vector.max_index(out=Iall[:, 8 * r:8 * r + 8],
                                    in_max=Vall[:, 8 * r:8 * r + 8], in_values=Ds)
                if r < 2:
                    nc.vector.match_replace(out=Ds,
                                            in_to_replace=Vall[:, 8 * r:8 * r + 8],
                                            in_values=Ds, imm_value=NEG)

            # idx roundtrip into the 16-wrapped dma_gather layout
            idxd = pdram.tile([nki], I16, tag="idxd")
            nc.sync.dma_start(
                out=idxd.rearrange("(j p) -> p j", p=128),
                in_=Iall[:, 0:K].bitcast(I16))
            idxw = pw.tile([128, nki // 16], I16, tag="idxw")
            idxd_rs = idxd.rearrange("(s r) -> r s", r=16)
            for g in range(8):
                nc.sync.dma_start(out=idxw[16 * g:16 * (g + 1), :], in_=idxd_rs)

            # gather [x_m ; exp(A_m)] rows for the 20 neighbors of each query
            Tg = pw.tile([128, K, TWO_C], F32, tag="Tg")
            n_split = 4
            sl = K // n_split
            ssl = nki // 16 // n_split
            for qn in range(n_split):
                nc.gpsimd.dma_gather(
                    out_ap=Tg[:, qn * sl:(qn + 1) * sl, :],
                    in_ap=TP,
                    idxs_ap=idxw[:, qn * ssl:(qn + 1) * ssl],
                    num_idxs=nki // n_split,
                    num_idxs_reg=nki // n_split,
                    elem_size=TWO_C,
                    queue_num=qn,
                )
            tg_slots[t] = Tg

        def tile_back(t):
            q0 = t * QT
            Tg = tg_slots.pop(t)
            # gated message passing (softmax gate, within-tile reductions)
            Eg = Tg[:, :, C:TWO_C].rearrange("p j c -> p c j")
            Xg = Tg[:, :, 0:C].rearrange("p j c -> p c j")
            S0 = pw.tile([128, C], F32, tag="S0")
            nc.vector.tensor_reduce(S0, Eg, axis=AX.X, op=ALU.add)
            Pt = pw.tile([128, C, K], F32, tag="Pt")
            nc.vector.tensor_mul(Pt, Eg, Xg)
            EX = pw.tile([128, C], F32, tag="EX")
            nc.vector.tensor_reduce(EX, Pt, axis=AX.X, op=ALU.add)
            R = pw.tile([128, C], F32, tag="R")
            nc.vector.reciprocal(R, S0)
            Gq = pw.tile([128, C], F32, tag="Gq")
            nc.vector.tensor_mul(Gq, EX, R)

            pg = pss.tile([C, 128], F32, tag="pg")
            nc.tensor.transpose(pg, Gq, ident)
            Gs = pw.tile([C, 128], F32, tag="Gs")
            nc.scalar.copy(Gs, pg)
            po = pss.tile([128, 128], F32, tag="po")
            nc.tensor.matmul(po, W2as, Gs, start=True, stop=True)
            O2t = pw.tile([128, 128], F32, tag="O2t")
            nc.sync.dma_start(out=O2t, in_=O2D[:, q0:q0 + QT])
            Outs = pw.tile([128, 128], F32, tag="Outs")
            nc.vector.tensor_add(Outs, po, O2t)
            nc.sync.dma_start(out=OUT[:, q0:q0 + QT], in_=Outs)

        for t in range(n_tiles):
            tile_front(t)
            if t > 0:
                tile_back(t - 1)
        if n_tiles > 0:
            tile_back(n_tiles - 1)


_CACHE = {}


def _get_compiled():
    if "nc" in _CACHE:
        return _CACHE["nc"]
    nc = bacc.Bacc("TRN2", target_bir_lowering=False, debug=False,
                   num_devices=N_CORES, num_swdge_queues=4)
    ins = {
        "XB": nc.dram_tensor("XB", [C, N], F32, kind="ExternalInput").ap(),
        "XQA": nc.dram_tensor("XQA", [C + 1, Q], F32, kind="ExternalInput").ap(),
        "XT": nc.dram_tensor("XT", [N, C], F32, kind="ExternalInput").ap(),
        "W1AAT": nc.dram_tensor("W1AAT", [C, C], F32, kind="ExternalInput").ap(),
        "W2AT": nc.dram_tensor("W2AT", [C, O], F32, kind="ExternalInput").ap(),
        "W2BT": nc.dram_tensor("W2BT", [C, O], F32, kind="ExternalInput").ap(),
    }
    outs = {"OUT": nc.dram_tensor("OUT", [O, Q], F32, kind="ExternalOutput").ap()}
    with tile.TileContext(nc) as tc:
        _build(tc, outs, ins, n=N, q=Q)
    nc.compile()
    nc.m = get_hw_module(nc.m)
    _CACHE["nc"] = nc
    return nc


def host_prepare(x_full, W1, W2):
    x_full = np.asarray(x_full, dtype=np.float32)
    W1 = np.asarray(W1, dtype=np.float32)
    W2 = np.asarray(W2, dtype=np.float32)
    per_batch = N_CORES // B
    in_maps = []
    W1AAT = np.ascontiguousarray(W1[:C, :C].T)
    W2AT = np.ascontiguousarray(W2[:, :C].T)
    W2BT = np.ascontiguousarray(W2[:, C:].T)
    for core in range(N_CORES):
        b = core // per_batch
        h = core % per_batch
        xb = np.ascontiguousarray(x_full[b])
        xq = xb[:, h * Q:(h + 1) * Q]
        xqa = np.empty((C + 1, Q), np.float32)
        xqa[:C] = 2.0 * xq
        xqa[C] = -1.0
        in_maps.append({
            "XB": xb,
            "XQA": np.ascontiguousarray(xqa),
            "XT": np.ascontiguousarray(xb.T),
            "W1AAT": W1AAT,
            "W2AT": W2AT,
            "W2BT": W2BT,
        })
    return in_maps


def run_cores(in_maps, trace=False, **kwargs):
    nc = _get_compiled()
    return bass_utils.run_bass_kernel_spmd(
        nc, in_maps, core_ids=list(range(N_CORES)), trace=trace, **kwargs)


def _get_runner():
    """Cached jitted shard_map executable (compile once, run many)."""
    if "runner" in _CACHE:
        return _CACHE["runner"]
    import jax
    from jax.sharding import Mesh, PartitionSpec
    from jax.experimental.shard_map import shard_map

    nc = _get_compiled()
    bass2jax.install_neuronx_cc_hook()
    partition_name = nc.partition_id_tensor.name if nc.partition_id_tensor else None
    in_names, out_names, out_avals, zero_shapes = [], [], [], []
    for alloc in nc.m.functions[0].allocations:
        if not isinstance(alloc, mybir.MemoryLocationSet):
            continue
        name = alloc.memorylocations[0].name
        if alloc.kind == "ExternalInput":
            if name != partition_name:
                in_names.append(name)
        elif alloc.kind == "ExternalOutput":
            out_names.append(name)
            shape = tuple(alloc.tensor_shape)
            dt = mybir.dt.np(alloc.dtype)
            out_avals.append(jax.core.ShapedArray(shape, dt))
            zero_shapes.append((shape, dt))
    n_params = len(in_names)
    all_in_names = list(in_names) + list(out_names)
    if partition_name is not None:
        all_in_names.append(partition_name)

    def _body(*args):
        operands = list(args)
        if partition_name is not None:
            operands.append(bass2jax.partition_id_tensor())
        outs = bass2jax._bass_exec_p.bind(
            *operands,
            out_avals=tuple(out_avals),
            in_names=tuple(all_in_names),
            out_names=tuple(out_names),
            lowering_input_output_aliases=(),
            sim_require_finite=True,
            sim_require_nnan=True,
            nc=nc,
        )
        return tuple(outs)

    devices = jax.devices()[:N_CORES]
    mesh = Mesh(np.asarray(devices), ("core",))
    n_outs = len(out_names)
    sharded = jax.jit(
        shard_map(_body, mesh=mesh,
                  in_specs=(PartitionSpec("core"),) * (n_params + n_outs),
                  out_specs=(PartitionSpec("core"),) * n_outs,
                  check_rep=False),
        keep_unused=True,
    )

    def run(in_maps):
        per_core = [[np.asarray(m[nm]) for nm in in_names] for m in in_maps]
        concat_in = [np.concatenate([per_core[c][i] for c in range(N_CORES)], axis=0)
                     for i in range(n_params)]
        concat_zeros = [np.zeros((N_CORES * s[0], *s[1:]), dt)
                        for s, dt in zero_shapes]
        out_arrs = sharded(*concat_in, *concat_zeros)
        res = []
        for c in range(N_CORES):
            res.append({nm: np.asarray(out_arrs[i]).reshape(
                N_CORES, *out_avals[i].shape)[c] for i, nm in enumerate(out_names)})
        return res

    _CACHE["runner"] = run
    return run


def kernel(x, W1, W2, spiral_size):
    assert int(spiral_size) == K
    in_maps = host_prepare(x, W1, W2)
    results = _get_runner()(in_maps)
    out = np.empty((B, O, N), dtype=np.float32)
    per_batch = N_CORES // B
    for core in range(N_CORES):
        b = core // per_batch
        h = core % per_batch
        out[b, :, h * Q:(h + 1) * Q] = results[core]["OUT"]
    return out

